# revision 1
# baseline (speedup 1.0000x reference)
"""ContinuousWaveletTransform (Morlet bank, 32 scales) on 8 TRN2 cores.

Key structure exploited: the reference wavelet is w[k] = exp(-0.5 k^2) *
exp(i 2pi k / 6) (bandwidth=1), so the envelope underflows to exactly 0.0f
after k=14, and taps k>=7 are < 2.3e-11.  Every scale shares the SAME 7
significant taps; the scale only sets a per-channel delay wl_c in
{64,194,...,2014,2048} (17 distinct values).  The dense (64ch x 2048-tap)
convolution therefore collapses to:

    out[c, n] = sum_{k=0}^{6} w_c[k] * sigp[n + 2048 - wl_c + k]

with sigp = [zeros(2048), signal].  Folding (delay, tap) pairs into one
contraction axis of 17*7 = 119 <= 128 rows makes each 512-wide output block
a single K=119 matmul: lhsT[7d+k, c] = w_c[k] (if delay(c)==d else 0),
rhs[7d+k, n] = sigp[n0 + n + 2048 - delay_d + k].

Sharding: sequence-parallel over L.  Core r handles n in [512r, 512(r+1))
for all 4 batches and all 64 (re,im) channels.  The host builds the tiny
im2col rhs (1 MB/core) with numpy; each core runs: DMA in -> 4 matmuls
(K=119, M=64, N=512, one per batch) -> 4 PSUM->DRAM DMAs.
"""

import os
import numpy as np

import concourse.bacc as bacc
import concourse.bass as bass
from concourse import mybir
from concourse.bass_utils import run_bass_kernel_spmd

# ---------------------------------------------------------------- constants
B = 4
L = 4096
N_SCALES = 32
WLMAX = 2048
NCORES = 8
NBLK = L // NCORES          # 512 output columns per core
T = 7                       # taps kept per wavelet (tap 7 is 2.3e-11)
P = 128                     # partition count / contraction rows (119 used)
NCH = 2 * N_SCALES          # 64: [re x32, im x32]

# distinct per-scale window lengths (= delays), scales 0..15 distinct,
# scales 16..31 all clamp to L/2 = 2048
_WLS = [64, 194, 324, 454, 584, 714, 844, 974, 1104, 1234, 1364, 1494,
        1624, 1754, 1884, 2014] + [2048] * 16
DELAYS = _WLS[:16] + [2048]          # 17 distinct
NDELAY = len(DELAYS)                 # 17
K_ROWS = NDELAY * T                  # 119

# matmul dtype: "float32" (exact, 4 cyc/row) or "float32r" (1 cyc/row)
MM_DTYPE = os.environ.get("CWT_MM_DTYPE", "float32")


def _wavelet_taps():
    t = np.arange(T, dtype=np.float32)
    env = np.exp(-0.5 * t * t).astype(np.float32)
    ph = np.float32(2.0 * np.pi * 1.0 / 6.0) * t
    wr = (env * np.cos(ph)).astype(np.float32)
    wi = (env * np.sin(ph)).astype(np.float32)
    return wr, wi


def _build_lhsT():
    """[128, 64] stationary operand: row 7d+k, col c -> w_c[k]."""
    wr, wi = _wavelet_taps()
    lhsT = np.zeros((P, NCH), np.float32)
    for sc in range(N_SCALES):
        d = sc if sc < 16 else 16
        for k in range(T):
            lhsT[T * d + k, sc] = wr[k]
            lhsT[T * d + k, N_SCALES + sc] = wi[k]
    return lhsT


def _build_rhs_per_core(signal):
    """Per-core [128, B*512] moving operands (im2col over (delay, tap))."""
    sigp = np.zeros((B, WLMAX + L), np.float32)
    sigp[:, WLMAX:] = signal
    rhs_all = []
    for r in range(NCORES):
        rhs = np.zeros((P, B * NBLK), np.float32)
        for d in range(NDELAY):
            s0 = WLMAX + NBLK * r - DELAYS[d]
            for b in range(B):
                for k in range(T):
                    rhs[T * d + k, NBLK * b:NBLK * (b + 1)] = \
                        sigp[b, s0 + k: s0 + k + NBLK]
        rhs_all.append(rhs)
    return rhs_all


def _build_nc():
    dt_mm = getattr(mybir.dt, MM_DTYPE)
    nc = bacc.Bacc("TRN2", target_bir_lowering=False, debug=False,
                   num_devices=NCORES)
    # rhs layout: [b0 (512) | lhsT (64) | b1 | b2 | b3] so the wavelet bank
    # rides the first chunk's DMA (no separate 128-descriptor transfer)
    rhs_d = nc.dram_tensor("rhs", [P, B * NBLK + NCH], dt_mm,
                           kind="ExternalInput")
    out_d = nc.dram_tensor("out", [NCH, B * NBLK], mybir.dt.float32,
                           kind="ExternalOutput")

    half = B * NBLK // 2                      # 1024 cols per input half
    hp = P // 2                               # 64-partition split per queue
    with (
        nc.sbuf_tensor("rhs_sb", [P, B * NBLK + NCH], dt_mm) as rhs_sb,
        nc.sbuf_tensor("out_sb", [NCH, B * NBLK], mybir.dt.float32) as out_sb,
        nc.psum_tensor("acc", [NCH, B, NBLK], mybir.dt.float32) as acc,
        nc.semaphore("s_r0") as s_r0,
        nc.semaphore("s_rb") as s_rb,
        nc.semaphore("s_r1") as s_r1,
        nc.semaphore("s_cp") as s_cp,
        nc.semaphore("s_mm") as s_mm,
        nc.semaphore("s_out") as s_out,
        nc.Block() as block,
    ):
        # Input DMAs are issued BEFORE the Block (bare engine calls) so
        # they enter each engine's stream right after the bass preamble
        # (~ts 5 us) instead of after the Block entry (~ts 7.2 us) —
        # the transfers hide under the NEFF boot barrier. Spread across
        # all three DMA-capable engines (SP/ACT HWDGE ~120 GB/s each,
        # GpSimd queue similar); batch-aligned chunks gate the matmuls.
        c1 = NBLK + NCH                   # end of chunk0 (b0 + lhsT)
        nc.sync.dma_start(
            rhs_sb[:, 0:c1], rhs_d[:, 0:c1]).then_inc(s_r0, 16)
        nc.scalar.dma_start(
            rhs_sb[:, c1:c1 + NBLK], rhs_d[:, c1:c1 + NBLK]
        ).then_inc(s_rb, 16)
        nc.gpsimd.dma_start(
            rhs_sb[:, c1 + NBLK:c1 + 3 * NBLK],
            rhs_d[:, c1 + NBLK:c1 + 3 * NBLK],
        ).then_inc(s_r1, 16)

        @block.sync
        def _(sync):
            for h in range(2):
                sync.wait_ge(s_cp, 2 * (h + 1))
                sync.dma_start(
                    out_d[0:NCH // 2, bass.ts(h, half)],
                    out_sb[0:NCH // 2, bass.ts(h, half)],
                ).then_inc(s_out, 16)
            sync.wait_ge(s_out, 64)

        @block.scalar
        def _(scalar):
            for h in range(2):
                scalar.wait_ge(s_cp, 2 * (h + 1))
                scalar.dma_start(
                    out_d[NCH // 2:NCH, bass.ts(h, half)],
                    out_sb[NCH // 2:NCH, bass.ts(h, half)],
                ).then_inc(s_out, 16)

        @block.tensor
        def _(tensor):
            gates = [s_r0, s_rb, s_r1, s_r1]
            offs = [0, NBLK + NCH, NBLK + NCH + NBLK, NBLK + NCH + 2 * NBLK]
            lhsT_ap = rhs_sb[:, NBLK:NBLK + NCH]
            for b in range(B):
                tensor.wait_ge(gates[b], 16)
                nc.tensor.matmul(
                    acc[:, b, :], lhsT_ap,
                    rhs_sb[:, offs[b]:offs[b] + NBLK],
                    start=True, stop=True,
                ).then_inc(s_mm, 1)

        @block.vector
        def _(vector):
            for b in range(B):
                vector.wait_ge(s_mm, b + 1)
                vector.tensor_copy(
                    out_sb[:, bass.ts(b, NBLK)], acc[:, b, :]
                ).then_inc(s_cp, 1)

    nc.compile()
    return nc


_NC_CACHE = {}


def _get_nc():
    key = MM_DTYPE
    if key not in _NC_CACHE:
        _NC_CACHE[key] = _build_nc()
    return _NC_CACHE[key]


def run(signal, trace=False, **spmd_kwargs):
    """Returns (out complex64 (4,32,4096), BassKernelResults)."""
    signal = np.asarray(signal, dtype=np.float32)
    assert signal.shape == (B, L)
    nc = _get_nc()
    lhsT = _build_lhsT()
    rhs_all = _build_rhs_per_core(signal)
    packed = [np.concatenate(
        [r[:, :NBLK], lhsT, r[:, NBLK:]], axis=1) for r in rhs_all]
    in_maps = [{"rhs": packed[r]} for r in range(NCORES)]
    res = run_bass_kernel_spmd(nc, in_maps, core_ids=list(range(NCORES)),
                               trace=trace, **spmd_kwargs)
    out = np.empty((B, N_SCALES, L), np.complex64)
    for r in range(NCORES):
        o = res.results[r]["out"]                      # [64, B*512] f32
        o = o.reshape(NCH, B, NBLK)
        sl = slice(NBLK * r, NBLK * (r + 1))
        for b in range(B):
            out[b, :, sl] = o[:N_SCALES, b, :] + 1j * o[N_SCALES:, b, :]
    return out, res


def kernel(signal):
    out, _ = run(signal, trace=False)
    return out



# revision 3
# speedup vs baseline: 1.1485x; 1.1485x over previous
"""ContinuousWaveletTransform (Morlet bank, 32 scales) on 8 TRN2 cores.

Key structure exploited: the reference wavelet is w[k] = exp(-0.5 k^2) *
exp(i 2pi k / 6) (bandwidth=1), so the envelope underflows to exactly 0.0f
after k=14, and taps k>=7 are < 2.3e-11.  Every scale shares the SAME 7
significant taps; the scale only sets a per-channel delay wl_c in
{64,194,...,2014,2048} (17 distinct values).  The dense (64ch x 2048-tap)
convolution therefore collapses to:

    out[c, n] = sum_{k=0}^{6} w_c[k] * sigp[n + 2048 - wl_c + k]

with sigp = [zeros(2048), signal].  Folding (delay, tap) pairs into one
contraction axis of 17*7 = 119 <= 128 rows makes each 512-wide output block
a single K=119 matmul: lhsT[7d+k, c] = w_c[k] (if delay(c)==d else 0),
rhs[7d+k, n] = sigp[n0 + n + 2048 - delay_d + k].

Sharding: sequence-parallel over L.  Core r handles n in [512r, 512(r+1))
for all 4 batches and all 64 (re,im) channels.  The host builds the tiny
im2col rhs in bf16 (0.5 MB/core); each core runs: 4 input DMAs (per-batch
chunks on the two HWDGE engines) -> 4 bf16 matmuls (K=119, M=64, N=512,
even batches land in PSUM partitions 0-63, odd in 64-127 via tile_position)
-> 2 PSUM->SBUF cast-copies of [128, 512] -> 3 SBUF->DRAM output DMAs
(bf16), host casts back to fp32/complex64.

bf16 keeps worst-case relative error ~4e-3 (harness gate 2e-2); set
CWT_MM_DTYPE=float32 for the exact fp32 variant.
"""

import os
import numpy as np
import ml_dtypes

import concourse.bacc as bacc
import concourse.bass as bass
from concourse import mybir
from concourse.bass_utils import run_bass_kernel_spmd

# ---------------------------------------------------------------- constants
B = 4
L = 4096
N_SCALES = 32
WLMAX = 2048
NCORES = 8
NBLK = L // NCORES          # 512 output columns per core
T = 7                       # taps kept per wavelet (tap 7 is 2.3e-11)
P = 128                     # partition count / contraction rows (119 used)
NCH = 2 * N_SCALES          # 64: [re x32, im x32]

# distinct per-scale window lengths (= delays), scales 0..15 distinct,
# scales 16..31 all clamp to L/2 = 2048
_WLS = [64, 194, 324, 454, 584, 714, 844, 974, 1104, 1234, 1364, 1494,
        1624, 1754, 1884, 2014] + [2048] * 16
DELAYS = _WLS[:16] + [2048]          # 17 distinct
NDELAY = len(DELAYS)                 # 17
K_ROWS = NDELAY * T                  # 119

# matmul dtype: "bfloat16" (1 cyc/row, rel err ~4e-3) or "float32" (exact)
MM_DTYPE = os.environ.get("CWT_MM_DTYPE", "bfloat16")
_NP_DT = {"bfloat16": ml_dtypes.bfloat16, "float32": np.float32,
          "float32r": np.float32}


def _wavelet_taps():
    t = np.arange(T, dtype=np.float32)
    env = np.exp(-0.5 * t * t).astype(np.float32)
    ph = np.float32(2.0 * np.pi * 1.0 / 6.0) * t
    wr = (env * np.cos(ph)).astype(np.float32)
    wi = (env * np.sin(ph)).astype(np.float32)
    return wr, wi


def _build_lhsT():
    """[128, 64] stationary operand: row 7d+k, col c -> w_c[k]."""
    wr, wi = _wavelet_taps()
    lhsT = np.zeros((P, NCH), np.float32)
    for sc in range(N_SCALES):
        d = sc if sc < 16 else 16
        for k in range(T):
            lhsT[T * d + k, sc] = wr[k]
            lhsT[T * d + k, N_SCALES + sc] = wi[k]
    return lhsT


def _build_rhs_per_core(signal):
    """Per-core [128, B*512] moving operands (im2col over (delay, tap))."""
    sigp = np.zeros((B, WLMAX + L), np.float32)
    sigp[:, WLMAX:] = signal
    rhs_all = []
    for r in range(NCORES):
        rhs = np.zeros((P, B * NBLK), np.float32)
        for d in range(NDELAY):
            s0 = WLMAX + NBLK * r - DELAYS[d]
            for b in range(B):
                for k in range(T):
                    rhs[T * d + k, NBLK * b:NBLK * (b + 1)] = \
                        sigp[b, s0 + k: s0 + k + NBLK]
        rhs_all.append(rhs)
    return rhs_all


def _build_nc():
    dt_mm = getattr(mybir.dt, MM_DTYPE)
    dt_out = mybir.dt.bfloat16 if MM_DTYPE == "bfloat16" else mybir.dt.float32
    nc = bacc.Bacc("TRN2", target_bir_lowering=False, debug=False,
                   num_devices=NCORES)
    # rhs layout: [b0 (512) | lhsT (64) | b1 | b2 | b3] so the wavelet bank
    # rides the first chunk's DMA
    rhs_d = nc.dram_tensor("rhs", [P, B * NBLK + NCH], dt_mm,
                           kind="ExternalInput")
    out_d = nc.dram_tensor("out", [NCH, B * NBLK], dt_out,
                           kind="ExternalOutput")

    c1 = NBLK + NCH                       # end of chunk0 (b0 + lhsT)
    offs = [0, c1, c1 + NBLK, c1 + 2 * NBLK]   # rhs col base per batch
    with (
        nc.sbuf_tensor("rhs_sb", [P, B * NBLK + NCH], dt_mm) as rhs_sb,
        nc.sbuf_tensor("out_sb", [P, 2 * NBLK], dt_out) as out_sb,
        nc.psum_tensor("acc", [P, 2, NBLK], mybir.dt.float32) as acc,
        nc.semaphore("s_in0") as s_in0,
        nc.semaphore("s_in1") as s_in1,
        nc.semaphore("s_in2") as s_in2,
        nc.semaphore("s_in3") as s_in3,
        nc.semaphore("s_mm") as s_mm,
        nc.semaphore("s_cp") as s_cp,
        nc.semaphore("s_out") as s_out,
        nc.Block() as block,
    ):
        s_in = [s_in0, s_in1, s_in2, s_in3]
        # Input DMAs issued BEFORE the Block (bare engine calls) so they
        # enter the streams right after the bass preamble.  Per-batch
        # chunks on the two HWDGE engines; batch b gates matmul b.
        nc.sync.dma_start(
            rhs_sb[:, 0:c1], rhs_d[:, 0:c1]).then_inc(s_in0, 16)
        nc.scalar.dma_start(
            rhs_sb[:, offs[1]:offs[1] + NBLK],
            rhs_d[:, offs[1]:offs[1] + NBLK]).then_inc(s_in1, 16)
        nc.sync.dma_start(
            rhs_sb[:, offs[2]:offs[2] + NBLK],
            rhs_d[:, offs[2]:offs[2] + NBLK]).then_inc(s_in2, 16)
        nc.scalar.dma_start(
            rhs_sb[:, offs[3]:offs[3] + NBLK],
            rhs_d[:, offs[3]:offs[3] + NBLK]).then_inc(s_in3, 16)

        @block.sync
        def _(sync):
            # out chunk b: dst out_d[:, b*512:(b+1)*512],
            # src out_sb[64*(b%2):+64, (b//2)*512:+512]; b=0,2 on sync
            for b in (0, 2):
                sync.wait_ge(s_cp, b // 2 + 1)
                sync.dma_start(
                    out_d[:, bass.ts(b, NBLK)],
                    out_sb[NCH * (b % 2):NCH * (b % 2) + NCH,
                           bass.ts(b // 2, NBLK)],
                ).then_inc(s_out, 16)
            sync.wait_ge(s_out, 64)

        @block.scalar
        def _(scalar):
            for b in (1, 3):
                scalar.wait_ge(s_cp, b // 2 + 1)
                scalar.dma_start(
                    out_d[:, bass.ts(b, NBLK)],
                    out_sb[NCH * (b % 2):NCH * (b % 2) + NCH,
                           bass.ts(b // 2, NBLK)],
                ).then_inc(s_out, 16)

        @block.tensor
        def _(tensor):
            lhsT_ap = rhs_sb[:, NBLK:NBLK + NCH]
            for b in range(B):
                tensor.wait_ge(s_in[b], 16)
                nc.tensor.matmul(
                    acc[NCH * (b % 2):NCH * (b % 2) + NCH, b // 2, :],
                    lhsT_ap,
                    rhs_sb[:, offs[b]:offs[b] + NBLK],
                    start=True, stop=True,
                ).then_inc(s_mm, 1)

        @block.vector
        def _(vector):
            for h in range(2):
                vector.wait_ge(s_mm, 2 * (h + 1))
                vector.tensor_copy(
                    out_sb[:, bass.ts(h, NBLK)], acc[:, h, :]
                ).then_inc(s_cp, 1)

    nc.compile()
    return nc


_NC_CACHE = {}


def _get_nc():
    key = MM_DTYPE
    if key not in _NC_CACHE:
        _NC_CACHE[key] = _build_nc()
    return _NC_CACHE[key]


def run(signal, trace=False, **spmd_kwargs):
    """Returns (out complex64 (4,32,4096), BassKernelResults)."""
    signal = np.asarray(signal, dtype=np.float32)
    assert signal.shape == (B, L)
    nc = _get_nc()
    np_dt = _NP_DT[MM_DTYPE]
    lhsT = _build_lhsT()
    rhs_all = _build_rhs_per_core(signal)
    packed = [np.concatenate(
        [r[:, :NBLK], lhsT, r[:, NBLK:]], axis=1).astype(np_dt)
        for r in rhs_all]
    in_maps = [{"rhs": packed[r]} for r in range(NCORES)]
    res = run_bass_kernel_spmd(nc, in_maps, core_ids=list(range(NCORES)),
                               trace=trace, **spmd_kwargs)
    out = np.empty((B, N_SCALES, L), np.complex64)
    for r in range(NCORES):
        o = np.asarray(res.results[r]["out"], np.float32)  # [64, B*512]
        o = o.reshape(NCH, B, NBLK)
        sl = slice(NBLK * r, NBLK * (r + 1))
        for b in range(B):
            out[b, :, sl] = o[:N_SCALES, b, :] + 1j * o[N_SCALES:, b, :]
    return out, res


def kernel(signal):
    out, _ = run(signal, trace=False)
    return out
